# revision 22
# baseline (speedup 1.0000x reference)
"""Trainium2 Bass kernel for nn_ClassificationHead: LayerNorm -> Linear(1024,256) -> GELU -> Linear(256,2).

Data-parallel over 8 NeuronCores: each core processes 8192 rows of the
65536-row batch; the tiny weights are replicated. The host supplies each
core's shard pre-transposed in bf16, block-major ([128, NB, KC, 512]) so
each block DMA is one contiguous 8KB run per partition (layout-only prep).

Per-core pipeline, per 512-row block (4 tiles of 128 rows):
  1. One DMA loads the K-major block [128, KC, 512] bf16 (SP HWDGE ring;
     weights ride the ACT ring in parallel at startup).
  2. Per tile, TensorE runs 8 accumulating matmuls against W1aug
     ([W1' | ones] -> PSUM cols 0:256 = x @ W1', col 256 = rowsum), plus a
     Gram matmul reusing the already-loaded stationary x-chunk
     (ldweights=False) into a second PSUM bank.
  3. ACT extracts -mu (rowsum * -1/D, Copy stays in the gelu activation
     table -- no table swaps); DVE extracts sum(x^2) (Gram diagonal via
     identity-masked scalar_tensor_tensor accumulate). V = SS/D - mu^2 + eps;
     g = rsqrt(V) via bit-trick seed + one fused Newton step (DVE);
     rhat = sqrt(V) = V*g (no ACT Sqrt).
  4. The per-row stats (-mu, rhat) are flipped into rows with a TensorE
     transpose-mode matmul (PSUM shared with the Gram ring) + ACT evac
     (bf16) -- no DMA on the critical path.
  5. TensorE adds the rank-2 correction (-mu ox s1 + rhat ox c1) as the
     9th accumulation chunk, so after the GELU's per-partition scale g the
     PSUM holds exactly LN(x)@W1'+b1.
  6. ACT evaluates exact GELU with scale g -> bf16 h tile.
  7. h @ W2 (2 output cols) is two masked-reduce dot products per tile on
     DVE (bf16 scratch -> 2x mode), decoupled from the stats path.
  8. Output accumulates in SBUF [128, 64, 2] and one partition-contiguous
     DMA writes it back; the host un-permutes (layout-only).

The block loop is software-pipelined: block u's stats-dependent matmuls
(transpose/rank-2/GELU) are emitted after the first two tiles of block
u+1, and its h@W2 dots after block u+1's stats chain, so the PE streams
continuously while DVE/ACT finish the per-block stats.

Host-side weight folding (tiny, O(1MB)): W1' = ln_w[:,None]*W1,
s1 = colsum(W1'), c1 = ln_b@W1 + b1.
"""
import sys

sys.path.insert(0, "/opt/trn_rl_repo")
sys.path.insert(0, "/root/.axon_site")

import numpy as np
import ml_dtypes

N_CORES = 8
BATCH = 65536
D = 1024
H = 256
OUT = 2
RPC = BATCH // N_CORES  # rows per core
NT = RPC // 128         # 128-row tiles per core
KC = D // 128           # contraction chunks
G = 4                   # tiles per block (512 rows)
NB = NT // G            # blocks per core
EPS = 1e-5
MAGIC = 0x5F3759DF

USE_COLTILE = False      # col-tiled gram (tile_position) -> compact [128,32] diag
NEWTON_ITERS = 1

_cache = {}


def _bf16(a):
    return np.asarray(a, dtype=ml_dtypes.bfloat16)


def _build(rpc=RPC):
    import concourse.bacc as bacc
    from concourse.tile_rust import add_dep_helper
    import concourse.mybir as mybir
    from concourse import tile

    f32 = mybir.dt.float32
    i32 = mybir.dt.int32
    bf16 = mybir.dt.bfloat16
    AF = mybir.ActivationFunctionType
    ALU = mybir.AluOpType

    nc = bacc.Bacc(None, target_bir_lowering=False, debug=False)

    nb = rpc // 128 // G
    xt_in = nc.dram_tensor("xt", [128, nb, KC, G * 128], bf16, kind="ExternalInput")
    w1_in = nc.dram_tensor("w1aug", [128, KC, H + 1], bf16, kind="ExternalInput")
    sc_in = nc.dram_tensor("screp", [2 * G, G, H + 1], bf16, kind="ExternalInput")
    w2_in = nc.dram_tensor("w2rep", [128, OUT, H], bf16, kind="ExternalInput")
    b2_in = nc.dram_tensor("b2g", [128, G * OUT], f32, kind="ExternalInput")
    idf_in = nc.dram_tensor("identf", [128, 128], f32, kind="ExternalInput")
    msk_in = nc.dram_tensor("mask32", [128, 32], f32, kind="ExternalInput")
    y_out = nc.dram_tensor("y", [128, rpc // 128, OUT], f32, kind="ExternalOutput")

    GW = 32 if USE_COLTILE else 128   # gram psum width

    with tile.TileContext(nc) as tc:
        with (
            tc.tile_pool(name="wpool", bufs=1) as wp,
            tc.tile_pool(name="xtp", bufs=6) as xtp,
            tc.tile_pool(name="statp", bufs=2) as statp,
            tc.tile_pool(name="scrp", bufs=2) as scrp,
            tc.tile_pool(name="hbp", bufs=5) as hbp,
            tc.tile_pool(name="outp", bufs=1) as outp,
            tc.tile_pool(name="pszp", bufs=6, space="PSUM") as pszp,
            tc.tile_pool(name="psgp", bufs=2, space="PSUM") as psgp,
        ):
            w1sb = wp.tile([128, KC, H + 1], bf16)
            nc.scalar.dma_start(w1sb[:], w1_in[:])
            scsb = wp.tile([2 * G, G, H + 1], bf16)
            nc.scalar.dma_start(scsb[:], sc_in[:])
            w2sb = wp.tile([128, OUT, H], bf16)
            nc.scalar.dma_start(w2sb[:], w2_in[:])
            b2sb = wp.tile([128, G * OUT], f32)
            nc.scalar.dma_start(b2sb[:], b2_in[:])
            idfsb = wp.tile([128, 128], f32)
            nc.scalar.dma_start(idfsb[:], idf_in[:])
            msksb = wp.tile([128, 32], f32)
            nc.scalar.dma_start(msksb[:], msk_in[:])

            nt = rpc // 128
            outsb = outp.tile([128, nt, OUT], f32)

            def emit_tile(u, q, xtg, BM, SS):
                """Mains + gram matmuls + per-tile extractions for tile q."""
                rs = q * 128
                pszg = pszp.tile([128, H + 1], f32, tag="pszg")
                psg = psgp.tile([128, GW], f32, tag="psg")
                for k in range(KC):
                    mm1 = nc.tensor.matmul(
                        pszg[:, 0 : H + 1], xtg[:, k, rs : rs + 128],
                        w1sb[:, k, :], start=(k == 0), stop=False,
                    )
                    if USE_COLTILE:
                        for c in range(4):
                            mmg = nc.tensor.matmul(
                                psg[32 * c : 32 * (c + 1), :],
                                xtg[:, k, rs + 32 * c : rs + 32 * (c + 1)],
                                xtg[:, k, rs + 32 * c : rs + 32 * (c + 1)],
                                start=(k == 0), stop=(k == KC - 1),
                                tile_position=(0, 32 * c),
                            )
                            mmg.ins.ldweights = False
                            add_dep_helper(mm1.ins, mmg.ins, False, "gram strip")
                    else:
                        mmg = nc.tensor.matmul(
                            psg[:],
                            xtg[:, k, rs : rs + 128], xtg[:, k, rs : rs + 128],
                            start=(k == 0), stop=(k == KC - 1),
                        )
                        mmg.ins.ldweights = False
                        add_dep_helper(mm1.ins, mmg.ins, False, "gram reuse")
                # -mu straight into the stats pack (ACT Copy, gelu table set)
                nc.scalar.activation(
                    BM[:, q, 0:1], pszg[:, H : H + 1], AF.Copy,
                    bias=0.0, scale=-1.0 / D,
                )
                # sum(x^2) = Gram diagonal (DVE); mask[r, j] = (j == r % GW)
                mask = msksb[:] if USE_COLTILE else idfsb[:]
                scr = scrp.tile([128, GW], f32, tag="scr")
                nc.vector.scalar_tensor_tensor(
                    scr[:], mask, 1.0, psg[:],
                    ALU.mult, ALU.mult, accum_out=SS[:, q : q + 1],
                )
                return pszg

            def emit_stats(u, BM, SS):
                """V = SS/D + eps - mu^2 ; g = rsqrt(V) (Newton); rhat = V*g."""
                A1 = statp.tile([128, G], f32, tag="A1")
                nc.vector.tensor_scalar(A1[:], SS[:], 1.0 / D, EPS, ALU.mult, ALU.add)
                B = statp.tile([128, G], f32, tag="B")
                nc.vector.tensor_tensor(B[:], BM[:, :, 0], BM[:, :, 0], ALU.mult)
                V = statp.tile([128, G], f32, tag="V")
                nc.vector.tensor_tensor(V[:], A1[:], B[:], ALU.subtract)
                GG = statp.tile([128, G], f32, tag="GG")
                T = statp.tile([128, G], f32, tag="T")
                nc.vector.tensor_scalar(
                    T[:].bitcast(i32), V[:].bitcast(i32), 1, None,
                    ALU.logical_shift_right,
                )
                nc.vector.tensor_scalar(
                    GG[:].bitcast(i32), T[:].bitcast(i32), -1, MAGIC,
                    ALU.mult, ALU.add,
                )
                for _ in range(NEWTON_ITERS):
                    # T = Y*Y ; T = (V*-0.5)*T ; Y = (T+1.5)*Y   (fused stt x2)
                    nc.vector.tensor_tensor(T[:], GG[:], GG[:], ALU.mult)
                    nc.vector.scalar_tensor_tensor(
                        T[:], V[:], -0.5, T[:], ALU.mult, ALU.mult,
                    )
                    nc.vector.scalar_tensor_tensor(
                        GG[:], T[:], 1.5, GG[:], ALU.add, ALU.mult,
                    )
                # rhat = sqrt(V) = V * rsqrt(V): no ACT Sqrt, so no
                # activation-table swap on the hot path
                nc.vector.tensor_tensor(BM[:, :, 1], V[:], GG[:], ALU.mult)
                return GG

            def emit_finish_pe(u, pszs, GG, BM):
                """Transpose stats, rank-2 correction, GELU for block u."""
                hbs = []
                pst = psgp.tile([128, GW], f32, tag="psg")
                BMf = BM[:].rearrange("p q s -> p (q s)")
                nc.tensor.transpose(pst[0 : 2 * G, :], BMf[:], idfsb[:])
                BMT = scrp.tile([2 * G, 128], bf16, tag="BMT")
                nc.scalar.activation(BMT[:], pst[0 : 2 * G, :], AF.Copy)
                for q in range(G):
                    pszg = pszs[q]
                    nc.tensor.matmul(
                        pszg[:, 0 : H + 1], BMT[:],
                        scsb[:, q, :], start=False, stop=True,
                    )
                    hb = hbp.tile([128, H], bf16, tag="hb")
                    nc.scalar.activation(
                        hb[:], pszg[:, 0:H], AF.Gelu, bias=0.0,
                        scale=GG[:, q : q + 1],
                    )
                    hbs.append(hb)
                return hbs

            def emit_finish_dve(u, hbs, OB):
                """h@W2 dots + bias add for block u (off the stats path)."""
                for q in range(G):
                    for c in range(OUT):
                        scr2 = scrp.tile([128, H], bf16, tag=f"scr2_{c}")
                        nc.vector.scalar_tensor_tensor(
                            scr2[:], hbs[q][:], 1.0, w2sb[:, c, :],
                            ALU.mult, ALU.mult, accum_out=OB[:, q, c : c + 1],
                        )
                nc.vector.tensor_add(
                    outsb[:, u * G : (u + 1) * G, :].opt(),
                    OB[:].opt(), b2sb[:].rearrange("p (q c) -> p q c", c=OUT),
                )

            # Software-pipelined block loop: block u's stats-dependent matmuls
            # (finish) are emitted after the FIRST tile of block u+1, keeping
            # the PE streaming while the DVE stats chain completes. With psum
            # bufs=5 the single lookahead tile has a free bank, and tile q1 of
            # block u+1 only needs a bank freed by block u's gelu, which is
            # already emitted by then.
            prev = None
            for u in range(nb):
                xtg = xtp.tile([128, KC, G * 128], bf16, tag="xtg")
                nc.sync.dma_start(xtg[:], xt_in[:, u, :, :])
                BM = statp.tile([128, G, 2], f32, tag="BM")
                SS = statp.tile([128, G], f32, tag="SS")
                pszs = [emit_tile(u, 0, xtg, BM, SS)]
                pszs.append(emit_tile(u, 1, xtg, BM, SS))
                if prev is not None:
                    pu, ppszs, pGG, pBM, pOB = prev
                    phbs = emit_finish_pe(pu, ppszs, pGG, pBM)
                for q in range(2, G):
                    pszs.append(emit_tile(u, q, xtg, BM, SS))
                GG = emit_stats(u, BM, SS)
                if prev is not None:
                    emit_finish_dve(pu, phbs, pOB)
                OB = statp.tile([128, G, OUT], f32, tag="OB")
                prev = (u, pszs, GG, BM, OB)
            # Last block: interleave the w2 dots right behind each gelu so
            # the pipeline drain is as short as possible.
            pu, ppszs, pGG, pBM, pOB = prev
            pst = psgp.tile([128, GW], f32, tag="psg")
            BMf = pBM[:].rearrange("p q s -> p (q s)")
            nc.tensor.transpose(pst[0 : 2 * G, :], BMf[:], idfsb[:])
            BMT = scrp.tile([2 * G, 128], bf16, tag="BMT")
            nc.scalar.activation(BMT[:], pst[0 : 2 * G, :], AF.Copy)
            for q in range(G):
                nc.tensor.matmul(
                    ppszs[q][:, 0 : H + 1], BMT[:],
                    scsb[:, q, :], start=False, stop=True,
                )
            for q in range(G):
                hb = hbp.tile([128, H], bf16, tag="hb")
                nc.scalar.activation(
                    hb[:], ppszs[q][:, 0:H], AF.Gelu, bias=0.0,
                    scale=pGG[:, q : q + 1],
                )
                for c in range(OUT):
                    scr2 = scrp.tile([128, H], bf16, tag=f"scr2_{c}")
                    nc.vector.scalar_tensor_tensor(
                        scr2[:], hb[:], 1.0, w2sb[:, c, :],
                        ALU.mult, ALU.mult, accum_out=pOB[:, q, c : c + 1],
                    )
            nc.vector.tensor_add(
                outsb[:, pu * G : (pu + 1) * G, :].opt(),
                pOB[:].opt(), b2sb[:].rearrange("p (q c) -> p q c", c=OUT),
            )

            nc.sync.dma_start(y_out[:], outsb[:])

    nc.finalize()
    return nc


def _get_nc():
    if "nc" not in _cache:
        _cache["nc"] = _build()
    return _cache["nc"]


def _prep_weights(ln_w, ln_b, W1, b1, W2, b2):
    W1p = ln_w[:, None] * W1                      # [1024, 256]
    s1 = W1p.sum(axis=0)                          # [256]
    c1 = ln_b @ W1 + b1                           # [256]
    w1aug = np.concatenate([W1p, np.ones((D, 1), np.float32)], axis=1)  # ones col -> rowsum
    sc = np.zeros((2 * G, G, H + 1), np.float32)
    for q in range(G):
        sc[2 * q, q, 0:H] = s1
        sc[2 * q + 1, q, 0:H] = c1
    return {
        "w1aug": _bf16(w1aug.reshape(KC, 128, H + 1).transpose(1, 0, 2)),
        "screp": _bf16(sc),
        "w2rep": _bf16(np.broadcast_to(W2.T, (128, OUT, H))),
        "b2g": np.broadcast_to(np.tile(b2, G), (128, G * OUT)).astype(np.float32).copy(),
        "identf": np.eye(128, dtype=np.float32),
        "mask32": np.ascontiguousarray(np.tile(np.eye(32, dtype=np.float32), (4, 1))),
    }


def _make_in_maps(embedding, ln_w, ln_b, W1, b1, W2, b2):
    embedding = np.asarray(embedding, dtype=np.float32)
    weights = _prep_weights(
        np.asarray(ln_w, dtype=np.float32), np.asarray(ln_b, dtype=np.float32),
        np.asarray(W1, dtype=np.float32), np.asarray(b1, dtype=np.float32),
        np.asarray(W2, dtype=np.float32), np.asarray(b2, dtype=np.float32),
    )
    xb = _bf16(embedding)                        # bf16 cast (rounding only)
    in_maps = []
    for c in range(N_CORES):
        xt = np.ascontiguousarray(xb[c * RPC : (c + 1) * RPC].T)  # [D, RPC]
        # block-major: [128, NB, KC, 512] so each (partition, block) slice is
        # one contiguous 8KB DMA run
        xt2 = np.ascontiguousarray(
            xt.reshape(KC, 128, NB, G * 128).transpose(1, 2, 0, 3)
        )
        in_maps.append({"xt": xt2, **weights})
    return in_maps


def kernel(embedding, ln_w, ln_b, W1, b1, W2, b2):
    from concourse.bass_utils import run_bass_kernel_spmd

    in_maps = _make_in_maps(embedding, ln_w, ln_b, W1, b1, W2, b2)
    nc = _get_nc()
    res = run_bass_kernel_spmd(nc, in_maps, core_ids=list(range(N_CORES)))
    out = np.concatenate(
        [
            res.results[c]["y"].transpose(1, 0, 2).reshape(RPC, OUT)
            for c in range(N_CORES)
        ],
        axis=0,
    )
    return out.astype(np.float32)


# revision 23
# speedup vs baseline: 1.0225x; 1.0225x over previous
"""Trainium2 Bass kernel for nn_ClassificationHead: LayerNorm -> Linear(1024,256) -> GELU -> Linear(256,2).

Data-parallel over 8 NeuronCores: each core processes 8192 rows of the
65536-row batch; the tiny weights are replicated. The host supplies each
core's shard pre-transposed in bf16, block-major ([128, NB, KC, 512]) so
each block DMA is one contiguous 8KB run per partition (layout-only prep).

Per-core pipeline, per 512-row block (4 tiles of 128 rows):
  1. One DMA loads the K-major block [128, KC, 512] bf16 (SP HWDGE ring;
     weights ride the ACT ring in parallel at startup).
  2. Per tile, TensorE runs 8 accumulating matmuls against W1aug
     ([W1' | ones] -> PSUM cols 0:256 = x @ W1', col 256 = rowsum), plus a
     Gram matmul reusing the already-loaded stationary x-chunk
     (ldweights=False) into a second PSUM bank.
  3. ACT extracts -mu (rowsum * -1/D, Copy stays in the gelu activation
     table -- no table swaps); DVE extracts sum(x^2) (Gram diagonal via
     identity-masked scalar_tensor_tensor accumulate). V = SS/D - mu^2 + eps;
     g = rsqrt(V) via bit-trick seed + one fused Newton step (DVE);
     rhat = sqrt(V) = V*g (no ACT Sqrt).
  4. The per-row stats (-mu, rhat) are flipped into rows with a TensorE
     transpose-mode matmul (PSUM shared with the Gram ring) + ACT evac
     (bf16) -- no DMA on the critical path.
  5. TensorE adds the rank-2 correction (-mu ox s1 + rhat ox c1) as the
     9th accumulation chunk, so after the GELU's per-partition scale g the
     PSUM holds exactly LN(x)@W1'+b1.
  6. ACT evaluates exact GELU with scale g -> bf16 h tile.
  7. h @ W2 (2 output cols) is two masked-reduce dot products per tile on
     DVE (bf16 scratch -> 2x mode), decoupled from the stats path.
  8. Output accumulates in SBUF [128, 64, 2] and one partition-contiguous
     DMA writes it back; the host un-permutes (layout-only).

The block loop is software-pipelined: block u's stats-dependent matmuls
(transpose/rank-2/GELU) are emitted after the first two tiles of block
u+1, and its h@W2 dots after block u+1's stats chain, so the PE streams
continuously while DVE/ACT finish the per-block stats.

Host-side weight folding (tiny, O(1MB)): W1' = ln_w[:,None]*W1,
s1 = colsum(W1'), c1 = ln_b@W1 + b1.
"""
import sys

sys.path.insert(0, "/opt/trn_rl_repo")
sys.path.insert(0, "/root/.axon_site")

import numpy as np
import ml_dtypes

N_CORES = 8
BATCH = 65536
D = 1024
H = 256
OUT = 2
RPC = BATCH // N_CORES  # rows per core
NT = RPC // 128         # 128-row tiles per core
KC = D // 128           # contraction chunks
G = 4                   # tiles per block (512 rows)
NB = NT // G            # blocks per core
EPS = 1e-5
MAGIC = 0x5F3759DF

USE_COLTILE = False      # col-tiled gram (tile_position) -> compact [128,32] diag
NEWTON_ITERS = 1

_cache = {}


def _bf16(a):
    return np.asarray(a, dtype=ml_dtypes.bfloat16)


def _build(rpc=RPC):
    import concourse.bacc as bacc
    from concourse.tile_rust import add_dep_helper
    import concourse.mybir as mybir
    from concourse import tile

    f32 = mybir.dt.float32
    i32 = mybir.dt.int32
    bf16 = mybir.dt.bfloat16
    AF = mybir.ActivationFunctionType
    ALU = mybir.AluOpType

    nc = bacc.Bacc(None, target_bir_lowering=False, debug=False)

    nb = rpc // 128 // G
    xt_in = nc.dram_tensor("xt", [128, nb, KC, G * 128], bf16, kind="ExternalInput")
    w1_in = nc.dram_tensor("w1aug", [128, KC, H + 1], bf16, kind="ExternalInput")
    sc_in = nc.dram_tensor("screp", [2 * G, G, H + 1], bf16, kind="ExternalInput")
    w2_in = nc.dram_tensor("w2rep", [128, OUT, H], bf16, kind="ExternalInput")
    b2_in = nc.dram_tensor("b2g", [128, G * OUT], f32, kind="ExternalInput")
    idf_in = nc.dram_tensor("identf", [128, 128], f32, kind="ExternalInput")
    msk_in = nc.dram_tensor("mask32", [128, 32], f32, kind="ExternalInput")
    y_out = nc.dram_tensor("y", [128, rpc // 128, OUT], f32, kind="ExternalOutput")

    GW = 32 if USE_COLTILE else 128   # gram psum width

    with tile.TileContext(nc) as tc:
        with (
            tc.tile_pool(name="wpool", bufs=1) as wp,
            tc.tile_pool(name="xtp", bufs=6) as xtp,
            tc.tile_pool(name="statp", bufs=2) as statp,
            tc.tile_pool(name="scrp", bufs=2) as scrp,
            tc.tile_pool(name="hbp", bufs=5) as hbp,
            tc.tile_pool(name="outp", bufs=1) as outp,
            tc.tile_pool(name="pszp", bufs=6, space="PSUM") as pszp,
            tc.tile_pool(name="psgp", bufs=2, space="PSUM") as psgp,
        ):
            w1sb = wp.tile([128, KC, H + 1], bf16)
            nc.scalar.dma_start(w1sb[:], w1_in[:])
            scsb = wp.tile([2 * G, G, H + 1], bf16)
            nc.scalar.dma_start(scsb[:], sc_in[:])
            w2sb = wp.tile([128, OUT, H], bf16)
            nc.scalar.dma_start(w2sb[:], w2_in[:])
            b2sb = wp.tile([128, G * OUT], f32)
            nc.scalar.dma_start(b2sb[:], b2_in[:])
            idfsb = wp.tile([128, 128], f32)
            nc.scalar.dma_start(idfsb[:], idf_in[:])
            msksb = wp.tile([128, 32], f32)
            nc.scalar.dma_start(msksb[:], msk_in[:])

            nt = rpc // 128
            outsb = outp.tile([128, nt, OUT], f32)

            def emit_tile(u, q, xtg, BM, SS):
                """Mains + gram matmuls + per-tile extractions for tile q."""
                rs = q * 128
                pszg = pszp.tile([128, H + 1], f32, tag="pszg")
                psg = psgp.tile([128, GW], f32, tag="psg")
                for k in range(KC):
                    mm1 = nc.tensor.matmul(
                        pszg[:, 0 : H + 1], xtg[:, k, rs : rs + 128],
                        w1sb[:, k, :], start=(k == 0), stop=False,
                    )
                    if USE_COLTILE:
                        for c in range(4):
                            mmg = nc.tensor.matmul(
                                psg[32 * c : 32 * (c + 1), :],
                                xtg[:, k, rs + 32 * c : rs + 32 * (c + 1)],
                                xtg[:, k, rs + 32 * c : rs + 32 * (c + 1)],
                                start=(k == 0), stop=(k == KC - 1),
                                tile_position=(0, 32 * c),
                            )
                            mmg.ins.ldweights = False
                            add_dep_helper(mm1.ins, mmg.ins, False, "gram strip")
                    else:
                        mmg = nc.tensor.matmul(
                            psg[:],
                            xtg[:, k, rs : rs + 128], xtg[:, k, rs : rs + 128],
                            start=(k == 0), stop=(k == KC - 1),
                        )
                        mmg.ins.ldweights = False
                        add_dep_helper(mm1.ins, mmg.ins, False, "gram reuse")
                # -mu straight into the stats pack (ACT Copy, gelu table set)
                nc.scalar.activation(
                    BM[:, q, 0:1], pszg[:, H : H + 1], AF.Copy,
                    bias=0.0, scale=-1.0 / D,
                )
                # sum(x^2) = Gram diagonal (DVE); mask[r, j] = (j == r % GW)
                mask = msksb[:] if USE_COLTILE else idfsb[:]
                scr = scrp.tile([128, GW], f32, tag="scr")
                nc.vector.scalar_tensor_tensor(
                    scr[:], mask, 1.0, psg[:],
                    ALU.mult, ALU.mult, accum_out=SS[:, q : q + 1],
                )
                return pszg

            def emit_stats(u, BM, SS):
                """V = SS/D + eps - mu^2 ; g = rsqrt(V) (Newton); rhat = V*g."""
                A1 = statp.tile([128, G], f32, tag="A1")
                nc.vector.tensor_scalar(A1[:], SS[:], 1.0 / D, EPS, ALU.mult, ALU.add)
                B = statp.tile([128, G], f32, tag="B")
                nc.vector.tensor_tensor(B[:], BM[:, :, 0], BM[:, :, 0], ALU.mult)
                V = statp.tile([128, G], f32, tag="V")
                nc.vector.tensor_tensor(V[:], A1[:], B[:], ALU.subtract)
                GG = statp.tile([128, G], f32, tag="GG")
                T = statp.tile([128, G], f32, tag="T")
                nc.vector.tensor_scalar(
                    T[:].bitcast(i32), V[:].bitcast(i32), 1, None,
                    ALU.logical_shift_right,
                )
                nc.vector.tensor_scalar(
                    GG[:].bitcast(i32), T[:].bitcast(i32), -1, MAGIC,
                    ALU.mult, ALU.add,
                )
                for _ in range(NEWTON_ITERS):
                    # T = Y*Y ; T = (V*-0.5)*T ; Y = (T+1.5)*Y   (fused stt x2)
                    nc.vector.tensor_tensor(T[:], GG[:], GG[:], ALU.mult)
                    nc.vector.scalar_tensor_tensor(
                        T[:], V[:], -0.5, T[:], ALU.mult, ALU.mult,
                    )
                    nc.vector.scalar_tensor_tensor(
                        GG[:], T[:], 1.5, GG[:], ALU.add, ALU.mult,
                    )
                # rhat = sqrt(V) = V * rsqrt(V): no ACT Sqrt, so no
                # activation-table swap on the hot path
                nc.vector.tensor_tensor(BM[:, :, 1], V[:], GG[:], ALU.mult)
                return GG

            def emit_finish_pe(u, pszs, GG, BM):
                """Transpose stats, rank-2 correction, GELU for block u."""
                hbs = []
                pst = psgp.tile([128, GW], f32, tag="psg")
                BMf = BM[:].rearrange("p q s -> p (q s)")
                nc.tensor.transpose(pst[0 : 2 * G, :], BMf[:], idfsb[:])
                BMT = scrp.tile([2 * G, 128], bf16, tag="BMT")
                nc.scalar.activation(BMT[:], pst[0 : 2 * G, :], AF.Copy)
                for q in range(G):
                    pszg = pszs[q]
                    nc.tensor.matmul(
                        pszg[:, 0 : H + 1], BMT[:],
                        scsb[:, q, :], start=False, stop=True,
                    )
                    hb = hbp.tile([128, H], bf16, tag="hb")
                    nc.scalar.activation(
                        hb[:], pszg[:, 0:H], AF.Gelu, bias=0.0,
                        scale=GG[:, q : q + 1],
                    )
                    hbs.append(hb)
                return hbs

            def emit_finish_dve(u, hbs, OB):
                """h@W2 dots + bias add for block u (off the stats path)."""
                for q in range(G):
                    for c in range(OUT):
                        scr2 = scrp.tile([128, H], bf16, tag=f"scr2_{c}")
                        nc.vector.scalar_tensor_tensor(
                            scr2[:], hbs[q][:], 1.0, w2sb[:, c, :],
                            ALU.mult, ALU.mult, accum_out=OB[:, q, c : c + 1],
                        )
                nc.vector.tensor_add(
                    outsb[:, u * G : (u + 1) * G, :].opt(),
                    OB[:].opt(), b2sb[:].rearrange("p (q c) -> p q c", c=OUT),
                )

            # Software-pipelined block loop: block u's stats-dependent matmuls
            # (finish) are emitted after the FIRST tile of block u+1, keeping
            # the PE streaming while the DVE stats chain completes. With psum
            # bufs=5 the single lookahead tile has a free bank, and tile q1 of
            # block u+1 only needs a bank freed by block u's gelu, which is
            # already emitted by then.
            prev = None
            for u in range(nb):
                xtg = xtp.tile([128, KC, G * 128], bf16, tag="xtg")
                nc.sync.dma_start(xtg[:], xt_in[:, u, :, :])
                BM = statp.tile([128, G, 2], f32, tag="BM")
                SS = statp.tile([128, G], f32, tag="SS")
                pszs = [emit_tile(u, 0, xtg, BM, SS)]
                pszs.append(emit_tile(u, 1, xtg, BM, SS))
                if prev is not None:
                    pu, ppszs, pGG, pBM, pOB = prev
                    phbs = emit_finish_pe(pu, ppszs, pGG, pBM)
                for q in range(2, G):
                    pszs.append(emit_tile(u, q, xtg, BM, SS))
                GG = emit_stats(u, BM, SS)
                if prev is not None:
                    emit_finish_dve(pu, phbs, pOB)
                OB = statp.tile([128, G, OUT], f32, tag="OB")
                prev = (u, pszs, GG, BM, OB)
            pu, ppszs, pGG, pBM, pOB = prev
            phbs = emit_finish_pe(pu, ppszs, pGG, pBM)
            emit_finish_dve(pu, phbs, pOB)

            nc.sync.dma_start(y_out[:], outsb[:])

    nc.finalize()
    return nc


def _get_nc():
    if "nc" not in _cache:
        _cache["nc"] = _build()
    return _cache["nc"]


def _prep_weights(ln_w, ln_b, W1, b1, W2, b2):
    W1p = ln_w[:, None] * W1                      # [1024, 256]
    s1 = W1p.sum(axis=0)                          # [256]
    c1 = ln_b @ W1 + b1                           # [256]
    w1aug = np.concatenate([W1p, np.ones((D, 1), np.float32)], axis=1)  # ones col -> rowsum
    sc = np.zeros((2 * G, G, H + 1), np.float32)
    for q in range(G):
        sc[2 * q, q, 0:H] = s1
        sc[2 * q + 1, q, 0:H] = c1
    return {
        "w1aug": _bf16(w1aug.reshape(KC, 128, H + 1).transpose(1, 0, 2)),
        "screp": _bf16(sc),
        "w2rep": _bf16(np.broadcast_to(W2.T, (128, OUT, H))),
        "b2g": np.broadcast_to(np.tile(b2, G), (128, G * OUT)).astype(np.float32).copy(),
        "identf": np.eye(128, dtype=np.float32),
        "mask32": np.ascontiguousarray(np.tile(np.eye(32, dtype=np.float32), (4, 1))),
    }


def _make_in_maps(embedding, ln_w, ln_b, W1, b1, W2, b2):
    embedding = np.asarray(embedding, dtype=np.float32)
    weights = _prep_weights(
        np.asarray(ln_w, dtype=np.float32), np.asarray(ln_b, dtype=np.float32),
        np.asarray(W1, dtype=np.float32), np.asarray(b1, dtype=np.float32),
        np.asarray(W2, dtype=np.float32), np.asarray(b2, dtype=np.float32),
    )
    xb = _bf16(embedding)                        # bf16 cast (rounding only)
    in_maps = []
    for c in range(N_CORES):
        xt = np.ascontiguousarray(xb[c * RPC : (c + 1) * RPC].T)  # [D, RPC]
        # block-major: [128, NB, KC, 512] so each (partition, block) slice is
        # one contiguous 8KB DMA run
        xt2 = np.ascontiguousarray(
            xt.reshape(KC, 128, NB, G * 128).transpose(1, 2, 0, 3)
        )
        in_maps.append({"xt": xt2, **weights})
    return in_maps


def kernel(embedding, ln_w, ln_b, W1, b1, W2, b2):
    from concourse.bass_utils import run_bass_kernel_spmd

    in_maps = _make_in_maps(embedding, ln_w, ln_b, W1, b1, W2, b2)
    nc = _get_nc()
    res = run_bass_kernel_spmd(nc, in_maps, core_ids=list(range(N_CORES)))
    out = np.concatenate(
        [
            res.results[c]["y"].transpose(1, 0, 2).reshape(RPC, OUT)
            for c in range(N_CORES)
        ],
        axis=0,
    )
    return out.astype(np.float32)


# revision 24
# speedup vs baseline: 1.0439x; 1.0210x over previous
"""Trainium2 Bass kernel for nn_ClassificationHead: LayerNorm -> Linear(1024,256) -> GELU -> Linear(256,2).

Data-parallel over 8 NeuronCores: each core processes 8192 rows of the
65536-row batch; the tiny weights are replicated. The host supplies each
core's shard pre-transposed in bf16, block-major ([128, NB, KC, 512]) so
each block DMA is one contiguous 8KB run per partition (layout-only prep).

Per-core pipeline, per 512-row block (4 tiles of 128 rows):
  1. One DMA loads the K-major block [128, KC, 512] bf16 (SP HWDGE ring;
     weights ride the ACT ring in parallel at startup).
  2. Per tile, TensorE runs 8 accumulating matmuls against W1aug
     ([W1' | ones] -> PSUM cols 0:256 = x @ W1', col 256 = rowsum), plus a
     Gram matmul reusing the already-loaded stationary x-chunk
     (ldweights=False) into a second PSUM bank.
  3. ACT extracts -mu (rowsum * -1/D, Copy stays in the gelu activation
     table -- no table swaps); DVE extracts sum(x^2) (Gram diagonal via
     identity-masked scalar_tensor_tensor accumulate). V = SS/D - mu^2 + eps;
     g = rsqrt(V) via bit-trick seed + one fused Newton step (DVE);
     rhat = sqrt(V) = V*g (no ACT Sqrt).
  4. The per-row stats (-mu, rhat) are flipped into rows with a TensorE
     transpose-mode matmul (PSUM shared with the Gram ring) + ACT evac
     (bf16) -- no DMA on the critical path.
  5. TensorE adds the rank-2 correction (-mu ox s1 + rhat ox c1) as the
     9th accumulation chunk, so after the GELU's per-partition scale g the
     PSUM holds exactly LN(x)@W1'+b1.
  6. ACT evaluates exact GELU with scale g -> bf16 h tile.
  7. h @ W2 (2 output cols) is two masked-reduce dot products per tile on
     DVE (bf16 scratch -> 2x mode), decoupled from the stats path.
  8. Output accumulates in SBUF [128, 64, 2] and one partition-contiguous
     DMA writes it back; the host un-permutes (layout-only).

The block loop is software-pipelined: block u's stats-dependent matmuls
(transpose/rank-2/GELU) are emitted after the first two tiles of block
u+1, and its h@W2 dots after block u+1's stats chain, so the PE streams
continuously while DVE/ACT finish the per-block stats.

Host-side weight folding (tiny, O(1MB)): W1' = ln_w[:,None]*W1,
s1 = colsum(W1'), c1 = ln_b@W1 + b1.
"""
import sys

sys.path.insert(0, "/opt/trn_rl_repo")
sys.path.insert(0, "/root/.axon_site")

import numpy as np
import ml_dtypes

N_CORES = 8
BATCH = 65536
D = 1024
H = 256
OUT = 2
RPC = BATCH // N_CORES  # rows per core
NT = RPC // 128         # 128-row tiles per core
KC = D // 128           # contraction chunks
G = 4                   # tiles per block (512 rows)
NB = NT // G            # blocks per core
EPS = 1e-5
MAGIC = 0x5F3759DF

USE_COLTILE = False      # col-tiled gram (tile_position) -> compact [128,32] diag
NEWTON_ITERS = 1

_cache = {}


def _bf16(a):
    return np.asarray(a, dtype=ml_dtypes.bfloat16)


def _build(rpc=RPC):
    import concourse.bacc as bacc
    from concourse.tile_rust import add_dep_helper
    import concourse.mybir as mybir
    from concourse import tile

    f32 = mybir.dt.float32
    i32 = mybir.dt.int32
    bf16 = mybir.dt.bfloat16
    AF = mybir.ActivationFunctionType
    ALU = mybir.AluOpType

    nc = bacc.Bacc(None, target_bir_lowering=False, debug=False)

    nb = rpc // 128 // G
    xt_in = nc.dram_tensor("xt", [128, nb, G, KC, 128], bf16, kind="ExternalInput")
    w1_in = nc.dram_tensor("w1aug", [128, KC, H + 1], bf16, kind="ExternalInput")
    sc_in = nc.dram_tensor("screp", [2 * G, G, H + 1], bf16, kind="ExternalInput")
    w2_in = nc.dram_tensor("w2rep", [128, OUT, H], bf16, kind="ExternalInput")
    b2_in = nc.dram_tensor("b2g", [128, G * OUT], f32, kind="ExternalInput")
    idf_in = nc.dram_tensor("identf", [128, 128], f32, kind="ExternalInput")
    msk_in = nc.dram_tensor("mask32", [128, 32], f32, kind="ExternalInput")
    y_out = nc.dram_tensor("y", [128, rpc // 128, OUT], f32, kind="ExternalOutput")

    GW = 32 if USE_COLTILE else 128   # gram psum width

    with tile.TileContext(nc) as tc:
        with (
            tc.tile_pool(name="wpool", bufs=1) as wp,
            tc.tile_pool(name="xtp", bufs=6) as xtp,
            tc.tile_pool(name="statp", bufs=2) as statp,
            tc.tile_pool(name="scrp", bufs=2) as scrp,
            tc.tile_pool(name="hbp", bufs=5) as hbp,
            tc.tile_pool(name="outp", bufs=1) as outp,
            tc.tile_pool(name="pszp", bufs=6, space="PSUM") as pszp,
            tc.tile_pool(name="psgp", bufs=2, space="PSUM") as psgp,
        ):
            w1sb = wp.tile([128, KC, H + 1], bf16)
            nc.scalar.dma_start(w1sb[:], w1_in[:])
            idfsb = wp.tile([128, 128], f32)
            nc.scalar.dma_start(idfsb[:], idf_in[:])
            scsb = wp.tile([2 * G, G, H + 1], bf16)
            nc.scalar.dma_start(scsb[:], sc_in[:])
            w2sb = wp.tile([128, OUT, H], bf16)
            nc.scalar.dma_start(w2sb[:], w2_in[:])
            b2sb = wp.tile([128, G * OUT], f32)
            nc.scalar.dma_start(b2sb[:], b2_in[:])
            msksb = wp.tile([128, 32], f32)
            nc.scalar.dma_start(msksb[:], msk_in[:])

            nt = rpc // 128
            outsb = outp.tile([128, nt, OUT], f32)

            def emit_tile(u, q, xsl, BM, SS):
                """Mains + gram matmuls + per-tile extractions for tile q."""
                pszg = pszp.tile([128, H + 1], f32, tag="pszg")
                psg = psgp.tile([128, GW], f32, tag="psg")
                for k in range(KC):
                    mm1 = nc.tensor.matmul(
                        pszg[:, 0 : H + 1], xsl(k),
                        w1sb[:, k, :], start=(k == 0), stop=False,
                    )
                    mmg = nc.tensor.matmul(
                        psg[:],
                        xsl(k), xsl(k),
                        start=(k == 0), stop=(k == KC - 1),
                    )
                    mmg.ins.ldweights = False
                    add_dep_helper(mm1.ins, mmg.ins, False, "gram reuse")
                # -mu straight into the stats pack (ACT Copy, gelu table set)
                nc.scalar.activation(
                    BM[:, q, 0:1], pszg[:, H : H + 1], AF.Copy,
                    bias=0.0, scale=-1.0 / D,
                )
                # sum(x^2) = Gram diagonal (DVE); mask[r, j] = (j == r % GW)
                mask = msksb[:] if USE_COLTILE else idfsb[:]
                scr = scrp.tile([128, GW], f32, tag="scr")
                nc.vector.scalar_tensor_tensor(
                    scr[:], mask, 1.0, psg[:],
                    ALU.mult, ALU.mult, accum_out=SS[:, q : q + 1],
                )
                return pszg

            def emit_stats(u, BM, SS):
                """V = SS/D + eps - mu^2 ; g = rsqrt(V) (Newton); rhat = V*g."""
                A1 = statp.tile([128, G], f32, tag="A1")
                nc.vector.tensor_scalar(A1[:], SS[:], 1.0 / D, EPS, ALU.mult, ALU.add)
                B = statp.tile([128, G], f32, tag="B")
                nc.vector.tensor_tensor(B[:], BM[:, :, 0], BM[:, :, 0], ALU.mult)
                V = statp.tile([128, G], f32, tag="V")
                nc.vector.tensor_tensor(V[:], A1[:], B[:], ALU.subtract)
                GG = statp.tile([128, G], f32, tag="GG")
                T = statp.tile([128, G], f32, tag="T")
                nc.vector.tensor_scalar(
                    T[:].bitcast(i32), V[:].bitcast(i32), 1, None,
                    ALU.logical_shift_right,
                )
                nc.vector.tensor_scalar(
                    GG[:].bitcast(i32), T[:].bitcast(i32), -1, MAGIC,
                    ALU.mult, ALU.add,
                )
                for _ in range(NEWTON_ITERS):
                    # T = Y*Y ; T = (V*-0.5)*T ; Y = (T+1.5)*Y   (fused stt x2)
                    nc.vector.tensor_tensor(T[:], GG[:], GG[:], ALU.mult)
                    nc.vector.scalar_tensor_tensor(
                        T[:], V[:], -0.5, T[:], ALU.mult, ALU.mult,
                    )
                    nc.vector.scalar_tensor_tensor(
                        GG[:], T[:], 1.5, GG[:], ALU.add, ALU.mult,
                    )
                # rhat = sqrt(V) = V * rsqrt(V): no ACT Sqrt, so no
                # activation-table swap on the hot path
                nc.vector.tensor_tensor(BM[:, :, 1], V[:], GG[:], ALU.mult)
                return GG

            def emit_finish_pe(u, pszs, GG, BM):
                """Transpose stats, rank-2 correction, GELU for block u."""
                hbs = []
                pst = psgp.tile([128, GW], f32, tag="psg")
                BMf = BM[:].rearrange("p q s -> p (q s)")
                nc.tensor.transpose(pst[0 : 2 * G, :], BMf[:], idfsb[:])
                BMT = scrp.tile([2 * G, 128], bf16, tag="BMT")
                nc.scalar.activation(BMT[:], pst[0 : 2 * G, :], AF.Copy)
                for q in range(G):
                    pszg = pszs[q]
                    nc.tensor.matmul(
                        pszg[:, 0 : H + 1], BMT[:],
                        scsb[:, q, :], start=False, stop=True,
                    )
                    hb = hbp.tile([128, H], bf16, tag="hb")
                    nc.scalar.activation(
                        hb[:], pszg[:, 0:H], AF.Gelu, bias=0.0,
                        scale=GG[:, q : q + 1],
                    )
                    hbs.append(hb)
                return hbs

            def emit_finish_dve(u, hbs, OB):
                """h@W2 dots + bias add for block u (off the stats path)."""
                for q in range(G):
                    for c in range(OUT):
                        scr2 = scrp.tile([128, H], bf16, tag=f"scr2_{c}")
                        nc.vector.scalar_tensor_tensor(
                            scr2[:], hbs[q][:], 1.0, w2sb[:, c, :],
                            ALU.mult, ALU.mult, accum_out=OB[:, q, c : c + 1],
                        )
                nc.vector.tensor_add(
                    outsb[:, u * G : (u + 1) * G, :].opt(),
                    OB[:].opt(), b2sb[:].rearrange("p (q c) -> p q c", c=OUT),
                )

            # Software-pipelined block loop: block u's stats-dependent matmuls
            # (finish) are emitted after the FIRST tile of block u+1, keeping
            # the PE streaming while the DVE stats chain completes. With psum
            # bufs=5 the single lookahead tile has a free bank, and tile q1 of
            # block u+1 only needs a bank freed by block u's gelu, which is
            # already emitted by then.
            prev = None
            for u in range(nb):
                if u == 0:
                    # per-tile DMAs: the first matmuls start after ~262KB
                    xts = []
                    for q in range(G):
                        xt0 = xtp.tile([128, KC, 128], bf16, tag=f"xt0_{q}")
                        nc.sync.dma_start(xt0[:], xt_in[:, 0, q, :, :])
                        xts.append(xt0)
                    slicers = [
                        (lambda k, t=xts[q]: t[:, k, :]) for q in range(G)
                    ]
                else:
                    xtg = xtp.tile([128, G, KC, 128], bf16, tag="xtg")
                    nc.sync.dma_start(xtg[:], xt_in[:, u, :, :, :])
                    slicers = [
                        (lambda k, t=xtg, qq=q: t[:, qq, k, :]) for q in range(G)
                    ]
                BM = statp.tile([128, G, 2], f32, tag="BM")
                SS = statp.tile([128, G], f32, tag="SS")
                pszs = [emit_tile(u, 0, slicers[0], BM, SS)]
                pszs.append(emit_tile(u, 1, slicers[1], BM, SS))
                if prev is not None:
                    pu, ppszs, pGG, pBM, pOB = prev
                    phbs = emit_finish_pe(pu, ppszs, pGG, pBM)
                for q in range(2, G):
                    pszs.append(emit_tile(u, q, slicers[q], BM, SS))
                GG = emit_stats(u, BM, SS)
                if prev is not None:
                    emit_finish_dve(pu, phbs, pOB)
                OB = statp.tile([128, G, OUT], f32, tag="OB")
                prev = (u, pszs, GG, BM, OB)
            pu, ppszs, pGG, pBM, pOB = prev
            phbs = emit_finish_pe(pu, ppszs, pGG, pBM)
            emit_finish_dve(pu, phbs, pOB)

            nc.sync.dma_start(y_out[:], outsb[:])

    nc.finalize()
    return nc


def _get_nc():
    if "nc" not in _cache:
        _cache["nc"] = _build()
    return _cache["nc"]


def _prep_weights(ln_w, ln_b, W1, b1, W2, b2):
    W1p = ln_w[:, None] * W1                      # [1024, 256]
    s1 = W1p.sum(axis=0)                          # [256]
    c1 = ln_b @ W1 + b1                           # [256]
    w1aug = np.concatenate([W1p, np.ones((D, 1), np.float32)], axis=1)  # ones col -> rowsum
    sc = np.zeros((2 * G, G, H + 1), np.float32)
    for q in range(G):
        sc[2 * q, q, 0:H] = s1
        sc[2 * q + 1, q, 0:H] = c1
    return {
        "w1aug": _bf16(w1aug.reshape(KC, 128, H + 1).transpose(1, 0, 2)),
        "screp": _bf16(sc),
        "w2rep": _bf16(np.broadcast_to(W2.T, (128, OUT, H))),
        "b2g": np.broadcast_to(np.tile(b2, G), (128, G * OUT)).astype(np.float32).copy(),
        "identf": np.eye(128, dtype=np.float32),
        "mask32": np.ascontiguousarray(np.tile(np.eye(32, dtype=np.float32), (4, 1))),
    }


def _make_in_maps(embedding, ln_w, ln_b, W1, b1, W2, b2):
    embedding = np.asarray(embedding, dtype=np.float32)
    weights = _prep_weights(
        np.asarray(ln_w, dtype=np.float32), np.asarray(ln_b, dtype=np.float32),
        np.asarray(W1, dtype=np.float32), np.asarray(b1, dtype=np.float32),
        np.asarray(W2, dtype=np.float32), np.asarray(b2, dtype=np.float32),
    )
    xb = _bf16(embedding)                        # bf16 cast (rounding only)
    in_maps = []
    for c in range(N_CORES):
        xt = np.ascontiguousarray(xb[c * RPC : (c + 1) * RPC].T)  # [D, RPC]
        # block-major: [128, NB, KC, 512] so each (partition, block) slice is
        # one contiguous 8KB DMA run
        xt2 = np.ascontiguousarray(
            xt.reshape(KC, 128, NB, G, 128).transpose(1, 2, 3, 0, 4)
        )
        in_maps.append({"xt": xt2, **weights})
    return in_maps


def kernel(embedding, ln_w, ln_b, W1, b1, W2, b2):
    from concourse.bass_utils import run_bass_kernel_spmd

    in_maps = _make_in_maps(embedding, ln_w, ln_b, W1, b1, W2, b2)
    nc = _get_nc()
    res = run_bass_kernel_spmd(nc, in_maps, core_ids=list(range(N_CORES)))
    out = np.concatenate(
        [
            res.results[c]["y"].transpose(1, 0, 2).reshape(RPC, OUT)
            for c in range(N_CORES)
        ],
        axis=0,
    )
    return out.astype(np.float32)
